# revision 38
# baseline (speedup 1.0000x reference)
"""Trainium2 Bass kernel for nn_AttentionHead_17042430231165.

out = softmax(min((x@wq.T+qb)@(x@wk.T+kb).T / 256, tri)) @ (x@wv.T+vb)
  x [32, 8192], wk/wq [256, 8192], wv [8192, 8192], tri [32, 32]

Sharding (8 cores):
  - wv rows (VAL) sharded: 1024 rows/core -> each core computes out[:, c*1024:(c+1)*1024]
  - wk/wq fully REPLICATED on every core (fp8-e4m3); scores and softmax
    computed locally. No collectives: profiling showed NRT's CC stream
    has ~60-85us of intrinsic bootstrap latency on this runtime (a
    1-byte AllGather triggered at 7.9us completed at 83.6us). A
    shared-HBM score exchange was also tried and failed: addr_space=
    "Shared" is per-rank collective staging, not a device-wide buffer
    reachable by plain local DMA.

Structure:
  - One HWDGE stream on the SP ring: x8 (0.25MB fp8) -> wkq (4.2MB fp8,
    4x1MB) -> x (0.5MB bf16) -> wv (16MB bf16, 32x0.5MB). The 0.5MB wv
    granularity matters: with 1MB DMAs, SDMA engine 15 (which runs ~15%
    slow) accumulates a serial backlog tail worth ~8us.
  - fp8 kq path: host prescales wk/wq/biases by S=1/std so values sit
    ~N(0,1) in e4m3; scores come out S^2-scaled and 1/(256 S^2) rides a
    runtime per-partition scale AP on the exp activation, with tri
    pre-scaled by 256 S^2. Costs ~1.2e-3 extra rel err.
  - PE: fp8 kq chain (x8 stationary, wkq moving 512 cols), then 128
    bf16 v-matmuls chase the wv tiles.
  - Scores locally: biases folded in via a 1-partition rank-1 matmul
    into the same PSUM group; kq [32,512] -> DVE 32x32 block transpose
    -> 8 accumulating [32dims x 32win] matmuls -> full scores.
  - Epilogue: one scalar-engine op computes the exp AND the softmax
    denominators (accum_out). e -> DVE transpose -> f32r -> two a@v
    matmuls -> fused (pu*rz)+vb -> stores split across both HWDGE
    rings. All score work is hidden under the wv stream.

Measured on trn2 (8 cores): 73.0-76.5us HW exec (core 0 profile),
rel err 3.6e-3 vs f32 reference (tolerance 2e-2). Earlier versions:
bf16 + runtime AllReduce 111-131us; bf16 local-scores 93.6us; fp8
1MB-DMA stream 82.9us.
"""
import sys

for _p in (
    "/root/.axon_site",
    "/root/.axon_site/_ro/trn_rl_repo",
    "/root/.axon_site/_ro/pypackages",
):
    if _p not in sys.path:
        sys.path.insert(0, _p)

import numpy as np
from ml_dtypes import bfloat16

from concourse import bacc, tile
from concourse import mybir
from concourse.bass_utils import run_bass_kernel_spmd

W = 32          # window (seq) size
IN = 8192       # in_size
KEY = 256       # key_size
VAL = 8192      # value_size
P = 128         # SBUF partitions
NCH = IN // P   # 64 contraction chunks
NCORES = 8
VSH = VAL // NCORES   # 1024 value dims per core
KQ = 2 * KEY    # 512 = full [k | q] projection width, replicated
SCALE = 1.0 / 256.0
NKQD = 8        # wkq stream DMA count (0.5MB each fp8, 8 chunks per tile)
KQC = NCH // NKQD
NVD = 32        # wv stream DMA count (0.5MB each, 2 chunks per tile)
VC = NCH // NVD
NT = 512        # moving free dim per matmul (fp32 max)

F32 = mybir.dt.float32
F32R = mybir.dt.float32r
BF16 = mybir.dt.bfloat16
F8 = mybir.dt.float8e4

_NC = None


def _build():
    global _NC
    if _NC is not None:
        return _NC
    nc = bacc.Bacc(None, target_bir_lowering=False, debug=False, num_devices=NCORES)

    X = nc.declare_dram_parameter("X", [P, NCH, W], BF16, isOutput=False)
    WKQ = nc.declare_dram_parameter("WKQ", [P, NCH, KQ], F8, isOutput=False)
    WV = nc.declare_dram_parameter("WV", [P, NCH, VSH], BF16, isOutput=False)
    KQB = nc.declare_dram_parameter("KQB", [1, KQ], F8, isOutput=False)
    VB = nc.declare_dram_parameter("VB", [1, VSH], F32, isOutput=False)
    TRI = nc.declare_dram_parameter("TRI", [W, W], F32, isOutput=False)
    SCL = nc.declare_dram_parameter("SCL", [W, 1], F32, isOutput=False)
    OUT = nc.declare_dram_parameter("out", [W, VSH], F32, isOutput=True)

    with tile.TileContext(nc) as tc:
        with (
            tc.tile_pool(name="const", bufs=1) as cpool,
            tc.tile_pool(name="kq", bufs=8) as kpool,
            tc.tile_pool(name="wv", bufs=16) as wpool,
            tc.tile_pool(name="small", bufs=1) as spool,
            tc.tile_pool(name="psum", bufs=1, space="PSUM") as ppool,
        ):
            # x (bf16) leads the SP stream; its fp8 twin for the kq chain
            # is derived on-device by one hidden DVE cast. Small constants
            # ride the otherwise-idle ACT ring.
            x_sb = cpool.tile([P, NCH, W], BF16)
            nc.sync.dma_start(out=x_sb[:], in_=X[:])
            x8_sb = cpool.tile([P, NCH, W], F8)
            nc.vector.tensor_copy(x8_sb[:], x_sb[:])
            kqb_sb = cpool.tile([1, KQ], F8)
            nc.scalar.dma_start(out=kqb_sb[:], in_=KQB[:])
            tri_sb = cpool.tile([W, W], F32)
            nc.scalar.dma_start(out=tri_sb[:], in_=TRI[:])
            scl_sb = cpool.tile([W, 1], F32)
            nc.scalar.dma_start(out=scl_sb[:], in_=SCL[:])
            vb_sb = cpool.tile([1, VSH], F32)
            nc.scalar.dma_start(out=vb_sb[:], in_=VB[:])
            ones1 = cpool.tile([1, W], F8)
            nc.vector.memset(ones1[:], 1.0)
            ones1f = cpool.tile([1, W], F32)
            nc.vector.memset(ones1f[:], 1.0)

            # kq' = S*(x @ [wk; wq].T + 1*[kb; qb])  -> [32, 512] natural rows
            pkq = ppool.tile([W, KQ], F32)
            for d in range(NKQD):
                kt = kpool.tile([P, KQC, KQ], F8, tag="kqstream")
                nc.sync.dma_start(out=kt[:], in_=WKQ[:, d * KQC:(d + 1) * KQC, :])
                for i in range(KQC):
                    c = d * KQC + i
                    nc.tensor.matmul(
                        pkq[:], x8_sb[:, c, :], kt[:, i, :],
                        start=(c == 0), stop=False,
                    )
            # bias via 1-partition rank-1 matmul into the same PSUM group
            nc.tensor.matmul(pkq[:], ones1[:], kqb_sb[:], start=False, stop=True,
                             skip_group_check=True)
            kq_sb = spool.tile([W, KQ], F32)
            nc.vector.tensor_copy(kq_sb[:], pkq[:])
            # 32x32 block transpose: block b holds kq[:, 32b:32b+32].T
            kqt = spool.tile([W, KQ], F32)
            nc.vector.transpose(kqt[:], kq_sb[:])
            # scores s[m,n] = sum_g q_g[:,m].T @ k_g[:,n], k blocks 0..7, q blocks 8..15
            ps = ppool.tile([W, W], F32)
            for g in range(8):
                nc.tensor.matmul(
                    ps[:], kqt[:, KEY + g * W:KEY + (g + 1) * W], kqt[:, g * W:(g + 1) * W],
                    start=(g == 0), stop=(g == 7),
                )
            S_sb = spool.tile([W, W], F32)
            nc.vector.tensor_copy(S_sb[:], ps[:])

            # preload vb into the output PSUM via rank-1 matmuls (hidden);
            # the tail's a@v then accumulates on top with start=False and
            # the final op is a plain PSUM->SBUF copy.
            pu0 = ppool.tile([W, NT], F32)
            pu1 = ppool.tile([W, NT], F32)
            nc.tensor.matmul(pu0[:], ones1f[:], vb_sb[:, 0:NT],
                             start=True, stop=False, skip_group_check=True)
            nc.tensor.matmul(pu1[:], ones1f[:], vb_sb[:, NT:VSH],
                             start=True, stop=False, skip_group_check=True)

            # v = x @ wv_c.T streamed over 64 contraction chunks (SP ring)
            pv0 = ppool.tile([W, NT], F32)
            pv1 = ppool.tile([W, NT], F32)
            for d in range(NVD):
                wt = wpool.tile([P, VC, VSH], BF16, tag="wvstream")
                nc.sync.dma_start(out=wt[:], in_=WV[:, d * VC:(d + 1) * VC, :])
                for i in range(VC):
                    c = d * VC + i
                    nc.tensor.matmul(
                        pv0[:], x_sb[:, c, :], wt[:, i, 0:NT],
                        start=(c == 0), stop=(c == NCH - 1),
                    )
                    nc.tensor.matmul(
                        pv1[:], x_sb[:, c, :], wt[:, i, NT:VSH],
                        start=(c == 0), stop=(c == NCH - 1),
                    )

            # softmax weights (hidden under the stream): e = exp(...) with
            # f32 row-sums via accum_out, a = e/Z applied up front, then aT.
            m_sb = spool.tile([W, W], F32)
            nc.vector.tensor_tensor(m_sb[:], S_sb[:], tri_sb[:], mybir.AluOpType.min)
            e_sb = spool.tile([W, W], F32)
            pz = spool.tile([W, 1], F32)
            nc.scalar.activation(
                e_sb[:], m_sb[:], mybir.ActivationFunctionType.Exp,
                scale=scl_sb[:], accum_out=pz[:],
            )
            rz = spool.tile([W, 1], F32)
            nc.vector.reciprocal(rz[:], pz[:])
            a_sb = spool.tile([W, W], F32)
            nc.vector.tensor_scalar_mul(a_sb[:], e_sb[:], rz[:])
            aTf = spool.tile([W, W], F32)
            nc.vector.transpose(aTf[:], a_sb[:])
            aT = spool.tile([W, W], F32R)
            nc.vector.tensor_copy(aT[:], aTf[:])

            # tail: v copies, a@v accumulated onto the vb preload, plain
            # copy out, stores overlapped on the two HWDGE rings
            for j, (pv, pu) in enumerate(((pv0, pu0), (pv1, pu1))):
                v_sb = spool.tile([W, NT], F32R, tag=f"v{j}")
                nc.vector.tensor_copy(v_sb[:], pv[:])
                nc.tensor.matmul(pu[:], aT[:], v_sb[:],
                                 start=False, stop=True, skip_group_check=True)
                o_sb = spool.tile([W, NT], F32, tag=f"o{j}")
                nc.vector.tensor_copy(o_sb[:], pu[:])
                eng = nc.scalar if j == 0 else nc.sync
                eng.dma_start(out=OUT[:, j * NT:(j + 1) * NT], in_=o_sb[:])

    nc.compile()
    _NC = nc
    return nc


def _swizzle(mat_t):
    """[rows=IN, cols] (transposed so IN is dim 0) -> bf16 [P, NCH, cols]."""
    rows, cols = mat_t.shape
    assert rows == IN
    return np.ascontiguousarray(
        mat_t.reshape(NCH, P, cols).transpose(1, 0, 2).astype(bfloat16))


def _swizzle8(mat_t):
    """[rows=IN, cols] -> fp8 e4m3 [P, NCH, cols] (clip to TRN max +-240)."""
    rows, cols = mat_t.shape
    assert rows == IN
    from ml_dtypes import float8_e4m3
    return np.ascontiguousarray(
        np.clip(mat_t, -240.0, 240.0)
        .reshape(NCH, P, cols).transpose(1, 0, 2).astype(float8_e4m3))


def _make_in_maps(x, wk_w, wk_b, wq_w, wq_b, wv_w, wv_b, tri):
    from ml_dtypes import float8_e4m3
    x = np.asarray(x, dtype=np.float32)
    xT = np.ascontiguousarray(x.T)
    X_dev = _swizzle(xT)
    # fp8 kq path: scale wk/wq (and biases) by S so values sit ~N(0,1) in
    # e4m3; scores come out as S^2 * raw and 1/(256*S^2) is applied inside
    # the exp (runtime scale tensor), with tri pre-scaled to match.
    wkq = np.concatenate([np.asarray(wk_w, dtype=np.float32),
                          np.asarray(wq_w, dtype=np.float32)], axis=0)
    S = 1.0 / max(float(np.std(wkq)), 1e-12)
    TRI = np.ascontiguousarray(
        np.asarray(tri, dtype=np.float32) * (256.0 * S * S))
    SCL = np.full((W, 1), SCALE / (S * S), dtype=np.float32)
    WKQ_dev = _swizzle8(np.ascontiguousarray(wkq.T) * S)
    KQB_dev = np.ascontiguousarray(np.clip(np.concatenate([
        np.asarray(wk_b, dtype=np.float32),
        np.asarray(wq_b, dtype=np.float32),
    ]) * S, -240.0, 240.0).reshape(1, KQ).astype(float8_e4m3))
    in_maps = []
    for c in range(NCORES):
        wv_sh = np.asarray(wv_w[c * VSH:(c + 1) * VSH, :], dtype=np.float32)
        in_maps.append({
            "X": X_dev,
            "WKQ": WKQ_dev,
            "WV": _swizzle(np.ascontiguousarray(wv_sh.T)),
            "KQB": KQB_dev,
            "VB": np.ascontiguousarray(
                np.asarray(wv_b[c * VSH:(c + 1) * VSH], dtype=np.float32)
                .reshape(1, VSH)),
            "TRI": TRI,
            "SCL": SCL,
        })
    return in_maps


def run(inputs, trace=False):
    """Build + run on 8 cores; returns (full_output, BassKernelResults)."""
    nc = _build()
    in_maps = _make_in_maps(**inputs)
    res = run_bass_kernel_spmd(
        nc, in_maps, core_ids=list(range(NCORES)), trace=trace,
    )
    out = np.concatenate([res.results[c]["out"] for c in range(NCORES)], axis=1)
    return out, res


def kernel(**inputs):
    out, _ = run(inputs, trace=False)
    return out


if __name__ == "__main__":
    rng = np.random.default_rng(0)
    ins = {
        "x": rng.standard_normal((W, IN), dtype=np.float32),
        "wk_w": rng.standard_normal((KEY, IN), dtype=np.float32) / 90.5,
        "wk_b": rng.standard_normal((KEY,), dtype=np.float32) / 90.5,
        "wq_w": rng.standard_normal((KEY, IN), dtype=np.float32) / 90.5,
        "wq_b": rng.standard_normal((KEY,), dtype=np.float32) / 90.5,
        "wv_w": rng.standard_normal((VAL, IN), dtype=np.float32) / 90.5,
        "wv_b": rng.standard_normal((VAL,), dtype=np.float32) / 90.5,
        "tri": ((np.tril(np.full((W, W), 2.0, dtype=np.float32)) - 1.0) * 1e5),
    }
    out = kernel(**ins)
    print("out", out.shape, out.dtype, np.abs(out).mean())
